# revision 33
# baseline (speedup 1.0000x reference)
"""MinLSTM Trainium2 kernel (8-core data-parallel over batch).

Math (per batch):
  preacts: F = x@Wf.T+bf, I = x@Wi.T+bi, Hp = x@Wh.T+bh      [T, H]
  sf=sigmoid(F), si=sigmoid(I)
  f_gate = sf/(sf+si)  (normalized gates; f+i=1)
  g(z) = max(sigmoid(z), z+0.5)
  h[0] = g(h_0);  h[t] = f_gate[t]*h[t-1] + (1-f_gate[t])*g(Hp[t])
Output: [T+1, H] per batch.

HW mapping per core (1 batch):
  - x transposed on PE into xT tiles [d,t] (fp32r) so matmuls contract d on
    partitions and produce [h, t] preact tiles; W rows transposed per
    h-block into fp32r lhsT tiles.
  - fp32r matmuls (1 cyc/row at N=512) accumulate preacts in PSUM.
  - ACT: sigmoids with fused per-partition bias, reading PSUM.
  - DVE: g via fused (Hp+b+0.5) max sh; normalization via reciprocal;
    v' = (f-1)*g fused; recurrence via tensor_tensor_scan (op1=subtract).
  - Scan output transposed back [h,t]->[t,h] on PE (delayed one h-block to
    keep PE dense) and DMA'd out.
"""
import sys

sys.path.insert(0, "/opt/trn_rl_repo")
import numpy as np

B, T, D, H = 8, 2048, 1024, 1024
N_CORES = 8
P = 128
TCH = 512
N_TC = T // TCH        # 4 time chunks
HB = H // P            # 8 h blocks
KD = D // P            # 8 contraction blocks
TS = T // P            # 16 time sub-tiles

_cache = {}


def _build_nc():
    import concourse.bacc as bacc
    import concourse.tile as tile
    from concourse import mybir
    from concourse.masks import make_identity
    from contextlib import ExitStack

    fp32 = mybir.dt.float32
    fp32r = mybir.dt.float32r
    ACT = mybir.ActivationFunctionType
    ALU = mybir.AluOpType

    nc = bacc.Bacc("TRN2", target_bir_lowering=False, debug=False,
                   num_devices=N_CORES)

    x = nc.dram_tensor("x", [T, D], fp32, kind="ExternalInput")
    h0 = nc.dram_tensor("h0", [1, H], fp32, kind="ExternalInput")
    Wf = nc.dram_tensor("Wf", [H, D], fp32, kind="ExternalInput")
    Wi = nc.dram_tensor("Wi", [H, D], fp32, kind="ExternalInput")
    Wh = nc.dram_tensor("Wh", [H, D], fp32, kind="ExternalInput")
    bf = nc.dram_tensor("bf", [H], fp32, kind="ExternalInput")
    bi = nc.dram_tensor("bi", [H], fp32, kind="ExternalInput")
    bh = nc.dram_tensor("bh", [H], fp32, kind="ExternalInput")
    y = nc.dram_tensor("y", [T + 1, H], fp32, kind="ExternalOutput")

    Ws = [Wf, Wi, Wh]

    with tile.TileContext(nc) as tc:
        with ExitStack() as ctx:
            consts = ctx.enter_context(tc.tile_pool(name="consts", bufs=1))
            xin_pool = ctx.enter_context(tc.tile_pool(name="xin", bufs=8))
            xt_pool = ctx.enter_context(tc.tile_pool(name="xt", bufs=1))
            win_pool = ctx.enter_context(tc.tile_pool(name="win", bufs=8))
            wt_pool = ctx.enter_context(tc.tile_pool(name="wt", bufs=2))
            gates = ctx.enter_context(tc.tile_pool(name="gates", bufs=2))
            hs_pool = ctx.enter_context(tc.tile_pool(name="hs", bufs=9))
            ost_pool = ctx.enter_context(tc.tile_pool(name="ost", bufs=8))
            mm_ps = ctx.enter_context(
                tc.tile_pool(name="mmps", bufs=5, space="PSUM"))
            wx_ps = ctx.enter_context(
                tc.tile_pool(name="wxps", bufs=3, space="PSUM"))

            # ---- constants: identity, biases, h0 ----
            idn = consts.tile([P, P], fp32, name="idn")
            make_identity(nc, idn[:, :])

            def load_col(name, src_ap):
                # gpsimd (SWDGE) keeps these scattered loads off the sync
                # queues that feed the startup x/W streams
                t = consts.tile([P, HB], fp32, name=name)
                nc.gpsimd.dma_start(
                    out=t, in_=src_ap.rearrange("(hb p) -> p hb", p=P))
                return t

            bf_t = load_col("bf_t", bf[:])
            bi_t = load_col("bi_t", bi[:])
            bh_t = load_col("bh_t", bh[:])
            h0_t = load_col("h0_t", h0[0, :])

            bhp5 = consts.tile([P, HB], fp32, name="bhp5")
            nc.vector.tensor_scalar_add(bhp5, bh_t, 0.5)
            sh0 = consts.tile([P, HB], fp32, name="sh0")
            nc.scalar.activation(sh0, h0_t, ACT.Sigmoid)
            g0 = consts.tile([P, HB], fp32, name="g0")
            # g0 = max(h0 + 0.5, sigmoid(h0))
            nc.vector.scalar_tensor_tensor(g0, h0_t, 0.5, sh0,
                                           op0=ALU.add, op1=ALU.max)
            nc.gpsimd.dma_start(
                out=y[0, :].rearrange("(hb p) -> p hb", p=P), in_=g0)

            # xt[kd][tc]: [128, TCH] fp32r tiles of x^T
            xt = [[None] * N_TC for _ in range(KD)]

            def emit_xT(tc_i):
                for kd in range(KD):
                    xt[kd][tc_i] = xt_pool.tile(
                        [P, TCH], fp32r, name=f"xt{kd}_{tc_i}",
                        tag=f"xt{kd}_{tc_i}")
                # half tiles give the transposes fine-grained DMA deps
                xins = []
                for j in range(4):
                    ts = tc_i * 4 + j
                    xlo = xin_pool.tile([P, D // 2], fp32,
                                        name=f"xin{ts}_lo", tag="xin")
                    nc.sync.dma_start(out=xlo,
                                      in_=x[ts * P:(ts + 1) * P, :D // 2])
                    xhi = xin_pool.tile([P, D // 2], fp32,
                                        name=f"xin{ts}_hi", tag="xin")
                    nc.sync.dma_start(out=xhi,
                                      in_=x[ts * P:(ts + 1) * P, D // 2:])
                    xins.append((xlo, xhi))
                # pack per-kd: psum bank gets the 4 time blocks of one kd so
                # a single [128,512] copy fills the whole xt[kd][tc] tile
                for kd in range(KD):
                    ps = wx_ps.tile([P, TCH], fp32, name=f"xtp{tc_i}_{kd}",
                                    tag="wxps")
                    half, kk = (0, kd) if kd < 4 else (1, kd - 4)
                    for j in range(4):
                        nc.tensor.transpose(
                            ps[:, j * P:(j + 1) * P],
                            xins[j][half][:, kk * P:(kk + 1) * P], idn)
                    if kd % 2:
                        nc.scalar.activation(xt[kd][tc_i], ps, ACT.Copy)
                    else:
                        nc.vector.tensor_copy(xt[kd][tc_i], ps)

            wt_tiles = [None] * HB
            _wprep_state = {}

            def emit_wprep_dma(hb):
                wt, wins = [], []
                for g in range(3):
                    halves = []
                    for q in range(2):
                        wh = win_pool.tile([P, D // 2], fp32,
                                           name=f"win{hb}_{g}_{q}", tag="win")
                        nc.sync.dma_start(
                            out=wh,
                            in_=Ws[g][hb * P:(hb + 1) * P,
                                      q * (D // 2):(q + 1) * (D // 2)])
                        halves.append(wh)
                    wins.append(halves)
                    wt.append(wt_pool.tile([P, D], fp32r, name=f"wt{hb}_{g}",
                                           tag=f"wt{g}"))
                wt_tiles[hb] = wt
                _wprep_state[hb] = wins

            def emit_wprep_piece(hb, piece):
                # piece in 0..5 -> (gate, half): 4 transposes + 1 copy.
                g, q = piece // 2, piece % 2
                win = _wprep_state[hb][g][q]
                wtg = wt_tiles[hb][g]
                ps = wx_ps.tile([P, TCH], fp32,
                                name=f"wtp{hb}_{g}_{q}", tag="wxps")
                for j in range(4):
                    nc.tensor.transpose(
                        ps[:, j * P:(j + 1) * P],
                        win[:, j * P:(j + 1) * P], idn)
                nc.scalar.activation(
                    wtg[:, q * TCH:(q + 1) * TCH], ps, ACT.Copy)

            def emit_wprep(hb):
                emit_wprep_dma(hb)
                for piece in range(6):
                    emit_wprep_piece(hb, piece)

            hs_tiles = [[None] * N_TC for _ in range(HB)]
            prev_hs_map = {}

            def emit_compute(hb, tcs=None, splits=1):
                wt = wt_tiles[hb]
                prev_hs, prev_end = prev_hs_map.get(hb, (None, TCH))
                W = TCH // splits
                for tc_i in (range(N_TC) if tcs is None else tcs):
                    pre = []
                    for g in range(3):
                        psg = mm_ps.tile([P, TCH], fp32,
                                         name=f"ps{hb}_{tc_i}_{g}", tag="mm")
                        for kd in range(KD):
                            nc.tensor.matmul(
                                psg,
                                wt[g][:, kd * P:(kd + 1) * P],
                                xt[kd][tc_i],
                                start=(kd == 0), stop=(kd == KD - 1))
                        pre.append(psg)

                    sf = gates.tile([P, TCH], fp32, name=f"sf{hb}_{tc_i}",
                                    tag="sf")
                    si = gates.tile([P, TCH], fp32, name=f"si{hb}_{tc_i}",
                                    tag="si")
                    sh = gates.tile([P, TCH], fp32, name=f"sh{hb}_{tc_i}",
                                    tag="sh")
                    gg = gates.tile([P, TCH], fp32, name=f"gg{hb}_{tc_i}",
                                    tag="gg")
                    den = gates.tile([P, TCH], fp32, name=f"den{hb}_{tc_i}",
                                     tag="den")
                    rec = gates.tile([P, TCH], fp32, name=f"rec{hb}_{tc_i}",
                                     tag="rec")
                    fg = gates.tile([P, TCH], fp32, name=f"fg{hb}_{tc_i}",
                                    tag="fg")
                    nv = gates.tile([P, TCH], fp32, name=f"nv{hb}_{tc_i}",
                                    tag="nv")
                    hs = hs_pool.tile([P, TCH], fp32, name=f"hs{hb}_{tc_i}",
                                      tag="hs")
                    for s in range(splits):
                        sl = slice(s * W, (s + 1) * W)
                        nc.scalar.activation(sf[:, sl], pre[0][:, sl],
                                             ACT.Sigmoid,
                                             bias=bf_t[:, hb:hb + 1])
                        nc.scalar.activation(si[:, sl], pre[1][:, sl],
                                             ACT.Sigmoid,
                                             bias=bi_t[:, hb:hb + 1])
                        nc.scalar.activation(sh[:, sl], pre[2][:, sl],
                                             ACT.Sigmoid,
                                             bias=bh_t[:, hb:hb + 1])
                        # g = max(Hp + bh + 0.5, sigmoid(Hp + bh))
                        nc.vector.scalar_tensor_tensor(
                            gg[:, sl], pre[2][:, sl], bhp5[:, hb:hb + 1],
                            sh[:, sl], op0=ALU.add, op1=ALU.max)
                        nc.vector.tensor_add(den[:, sl], sf[:, sl],
                                             si[:, sl])
                        nc.vector.reciprocal_approx_fast(rec[:, sl],
                                                         den[:, sl])
                        nc.vector.tensor_mul(fg[:, sl], sf[:, sl],
                                             rec[:, sl])
                        # nv = (f-1)*g (scan's op1=subtract adds (1-f)*g)
                        nc.vector.scalar_tensor_tensor(
                            nv[:, sl], fg[:, sl], 1.0, gg[:, sl],
                            op0=ALU.subtract, op1=ALU.mult)
                        init = (g0[:, hb:hb + 1]
                                if tc_i == 0 and s == 0
                                else prev_hs[:, prev_end - 1:prev_end])
                        nc.vector.tensor_tensor_scan(hs[:, sl], fg[:, sl],
                                                     nv[:, sl], init,
                                                     op0=ALU.mult,
                                                     op1=ALU.subtract)
                        prev_hs, prev_end = hs, sl.stop
                    hs_tiles[hb][tc_i] = hs
                prev_hs_map[hb] = (prev_hs, prev_end)

            def emit_out(hb, tcs=None):
                # DVE 32x32 block transpose + block-permuting DMA AP: the
                # in-block transpose happens on DVE, the block-position swap
                # happens in the DMA's access pattern (32-float runs).
                for tc_i in (range(N_TC) if tcs is None else tcs):
                    t0 = tc_i * TCH
                    hs = hs_tiles[hb][tc_i]
                    hst = ost_pool.tile([P, TCH], fp32,
                                        name=f"hst{hb}_{tc_i}", tag="ost")
                    nc.vector.transpose(hst, hs)
                    y_ap = y[1 + t0:1 + t0 + TCH, hb * P:(hb + 1) * P]
                    for A in range(4):
                        nc.sync.dma_start(
                            out=y_ap[:, A * 32:(A + 1) * 32].rearrange(
                                "(Bq u) v -> u Bq v", u=32),
                            in_=hst[A * 32:(A + 1) * 32, :].rearrange(
                                "u (Bq v) -> u Bq v", v=32))

            # ---- emission schedule (software-pipelined for PE density) ----
            # W-prep is emitted mid-way through the previous h-block so its
            # ACT copies drain before the next block's matmuls need them;
            # output transposes are delayed one h-block so their scan inputs
            # are long since ready when PE reaches them.
            emit_wprep(0)
            for tc_i in range(N_TC):
                emit_xT(tc_i)
                emit_compute(0, tcs=[tc_i])
                if tc_i == 0:
                    emit_wprep(1)
            for hb in range(1, HB):
                emit_compute(hb, tcs=[0])
                if hb + 1 < HB:
                    emit_wprep(hb + 1)
                if hb < HB - 1:
                    emit_compute(hb, tcs=[1, 2, 3])
                    emit_out(hb - 1)
                else:
                    emit_compute(hb, tcs=[1], splits=2)
                    emit_out(hb - 1, tcs=[0, 1])
                    emit_compute(hb, tcs=[2], splits=2)
                    emit_out(hb - 1, tcs=[2, 3])
                    emit_compute(hb, tcs=[3], splits=2)
                    emit_out(hb, tcs=[0, 1])
            emit_out(HB - 1, tcs=[2, 3])

    nc.compile()
    return nc


def _get_nc():
    if "nc" not in _cache:
        _cache["nc"] = _build_nc()
    return _cache["nc"]


def _run(inputs, trace=False, **kw):
    from concourse.bass_utils import run_bass_kernel_spmd

    nc = _get_nc()
    x = np.ascontiguousarray(inputs["x"], dtype=np.float32)
    h_0 = np.ascontiguousarray(inputs["h_0"], dtype=np.float32)
    shared = {
        "Wf": np.ascontiguousarray(inputs["Wf"], dtype=np.float32),
        "Wi": np.ascontiguousarray(inputs["Wi"], dtype=np.float32),
        "Wh": np.ascontiguousarray(inputs["Wh"], dtype=np.float32),
        "bf": np.ascontiguousarray(inputs["bf"], dtype=np.float32),
        "bi": np.ascontiguousarray(inputs["bi"], dtype=np.float32),
        "bh": np.ascontiguousarray(inputs["bh"], dtype=np.float32),
    }
    in_maps = []
    for b in range(B):
        m = {"x": x[b], "h0": h_0[b], **shared}
        in_maps.append(m)
    res = run_bass_kernel_spmd(nc, in_maps, list(range(N_CORES)),
                               trace=trace, **kw)
    out = np.stack([res.results[b]["y"] for b in range(B)], axis=0)
    return out, res


def kernel(**inputs) -> np.ndarray:
    out, _ = _run(inputs, trace=False)
    return out


# revision 34
# speedup vs baseline: 1.0320x; 1.0320x over previous
"""MinLSTM Trainium2 kernel (8-core data-parallel over batch).

Math (per batch):
  preacts: F = x@Wf.T+bf, I = x@Wi.T+bi, Hp = x@Wh.T+bh      [T, H]
  sf=sigmoid(F), si=sigmoid(I)
  f_gate = sf/(sf+si)  (normalized gates; f+i=1)
  g(z) = max(sigmoid(z), z+0.5)
  h[0] = g(h_0);  h[t] = f_gate[t]*h[t-1] + (1-f_gate[t])*g(Hp[t])
Output: [T+1, H] per batch.

HW mapping per core (1 batch):
  - x transposed on PE into xT tiles [d,t] (fp32r) so matmuls contract d on
    partitions and produce [h, t] preact tiles; W rows transposed per
    h-block into fp32r lhsT tiles.
  - fp32r matmuls (1 cyc/row at N=512) accumulate preacts in PSUM.
  - ACT: sigmoids with fused per-partition bias, reading PSUM.
  - DVE: g via fused (Hp+b+0.5) max sh; normalization via reciprocal;
    v' = (f-1)*g fused; recurrence via tensor_tensor_scan (op1=subtract).
  - Scan output transposed back [h,t]->[t,h] on PE (delayed one h-block to
    keep PE dense) and DMA'd out.
"""
import sys

sys.path.insert(0, "/opt/trn_rl_repo")
import numpy as np

B, T, D, H = 8, 2048, 1024, 1024
N_CORES = 8
P = 128
TCH = 512
N_TC = T // TCH        # 4 time chunks
HB = H // P            # 8 h blocks
KD = D // P            # 8 contraction blocks
TS = T // P            # 16 time sub-tiles

_cache = {}


def _build_nc():
    import concourse.bacc as bacc
    import concourse.tile as tile
    from concourse import mybir
    from concourse.masks import make_identity
    from contextlib import ExitStack

    fp32 = mybir.dt.float32
    fp32r = mybir.dt.float32r
    ACT = mybir.ActivationFunctionType
    ALU = mybir.AluOpType

    nc = bacc.Bacc("TRN2", target_bir_lowering=False, debug=False,
                   num_devices=N_CORES)

    x = nc.dram_tensor("x", [T, D], fp32, kind="ExternalInput")
    h0 = nc.dram_tensor("h0", [1, H], fp32, kind="ExternalInput")
    Wf = nc.dram_tensor("Wf", [H, D], fp32, kind="ExternalInput")
    Wi = nc.dram_tensor("Wi", [H, D], fp32, kind="ExternalInput")
    Wh = nc.dram_tensor("Wh", [H, D], fp32, kind="ExternalInput")
    bf = nc.dram_tensor("bf", [H], fp32, kind="ExternalInput")
    bi = nc.dram_tensor("bi", [H], fp32, kind="ExternalInput")
    bh = nc.dram_tensor("bh", [H], fp32, kind="ExternalInput")
    y = nc.dram_tensor("y", [T + 1, H], fp32, kind="ExternalOutput")

    Ws = [Wf, Wi, Wh]

    with tile.TileContext(nc) as tc:
        with ExitStack() as ctx:
            consts = ctx.enter_context(tc.tile_pool(name="consts", bufs=1))
            xin_pool = ctx.enter_context(tc.tile_pool(name="xin", bufs=8))
            xt_pool = ctx.enter_context(tc.tile_pool(name="xt", bufs=1))
            win_pool = ctx.enter_context(tc.tile_pool(name="win", bufs=8))
            wt_pool = ctx.enter_context(tc.tile_pool(name="wt", bufs=2))
            gates = ctx.enter_context(tc.tile_pool(name="gates", bufs=2))
            hs_pool = ctx.enter_context(tc.tile_pool(name="hs", bufs=9))
            ost_pool = ctx.enter_context(tc.tile_pool(name="ost", bufs=8))
            mm_ps = ctx.enter_context(
                tc.tile_pool(name="mmps", bufs=5, space="PSUM"))
            wx_ps = ctx.enter_context(
                tc.tile_pool(name="wxps", bufs=3, space="PSUM"))

            # ---- constants: identity, biases, h0 ----
            idn = consts.tile([P, P], fp32, name="idn")
            make_identity(nc, idn[:, :])

            def load_col(name, src_ap):
                # gpsimd (SWDGE) keeps these scattered loads off the sync
                # queues that feed the startup x/W streams
                t = consts.tile([P, HB], fp32, name=name)
                nc.gpsimd.dma_start(
                    out=t, in_=src_ap.rearrange("(hb p) -> p hb", p=P))
                return t

            bf_t = load_col("bf_t", bf[:])
            bi_t = load_col("bi_t", bi[:])
            bh_t = load_col("bh_t", bh[:])
            h0_t = load_col("h0_t", h0[0, :])

            bhp5 = consts.tile([P, HB], fp32, name="bhp5")
            nc.vector.tensor_scalar_add(bhp5, bh_t, 0.5)
            sh0 = consts.tile([P, HB], fp32, name="sh0")
            nc.scalar.activation(sh0, h0_t, ACT.Sigmoid)
            g0 = consts.tile([P, HB], fp32, name="g0")
            # g0 = max(h0 + 0.5, sigmoid(h0))
            nc.vector.scalar_tensor_tensor(g0, h0_t, 0.5, sh0,
                                           op0=ALU.add, op1=ALU.max)
            nc.gpsimd.dma_start(
                out=y[0, :].rearrange("(hb p) -> p hb", p=P), in_=g0)

            # xt[kd][tc]: [128, TCH] fp32r tiles of x^T
            xt = [[None] * N_TC for _ in range(KD)]

            def emit_xT(tc_i):
                for kd in range(KD):
                    xt[kd][tc_i] = xt_pool.tile(
                        [P, TCH], fp32r, name=f"xt{kd}_{tc_i}",
                        tag=f"xt{kd}_{tc_i}")
                # half tiles give the transposes fine-grained DMA deps
                xins = []
                for j in range(4):
                    ts = tc_i * 4 + j
                    xlo = xin_pool.tile([P, D // 2], fp32,
                                        name=f"xin{ts}_lo", tag="xin")
                    nc.sync.dma_start(out=xlo,
                                      in_=x[ts * P:(ts + 1) * P, :D // 2])
                    xhi = xin_pool.tile([P, D // 2], fp32,
                                        name=f"xin{ts}_hi", tag="xin")
                    nc.sync.dma_start(out=xhi,
                                      in_=x[ts * P:(ts + 1) * P, D // 2:])
                    xins.append((xlo, xhi))
                # pack per-kd: psum bank gets the 4 time blocks of one kd so
                # a single [128,512] copy fills the whole xt[kd][tc] tile
                for kd in range(KD):
                    ps = wx_ps.tile([P, TCH], fp32, name=f"xtp{tc_i}_{kd}",
                                    tag="wxps")
                    half, kk = (0, kd) if kd < 4 else (1, kd - 4)
                    for j in range(4):
                        nc.tensor.transpose(
                            ps[:, j * P:(j + 1) * P],
                            xins[j][half][:, kk * P:(kk + 1) * P], idn)
                    nc.scalar.activation(xt[kd][tc_i], ps, ACT.Copy)

            wt_tiles = [None] * HB
            _wprep_state = {}

            def emit_wprep_dma(hb):
                wt, wins = [], []
                for g in range(3):
                    halves = []
                    for q in range(2):
                        wh = win_pool.tile([P, D // 2], fp32,
                                           name=f"win{hb}_{g}_{q}", tag="win")
                        nc.sync.dma_start(
                            out=wh,
                            in_=Ws[g][hb * P:(hb + 1) * P,
                                      q * (D // 2):(q + 1) * (D // 2)])
                        halves.append(wh)
                    wins.append(halves)
                    wt.append(wt_pool.tile([P, D], fp32r, name=f"wt{hb}_{g}",
                                           tag=f"wt{g}"))
                wt_tiles[hb] = wt
                _wprep_state[hb] = wins

            def emit_wprep_piece(hb, piece):
                # piece in 0..5 -> (gate, half): 4 transposes + 1 copy.
                g, q = piece // 2, piece % 2
                win = _wprep_state[hb][g][q]
                wtg = wt_tiles[hb][g]
                ps = wx_ps.tile([P, TCH], fp32,
                                name=f"wtp{hb}_{g}_{q}", tag="wxps")
                for j in range(4):
                    nc.tensor.transpose(
                        ps[:, j * P:(j + 1) * P],
                        win[:, j * P:(j + 1) * P], idn)
                nc.scalar.activation(
                    wtg[:, q * TCH:(q + 1) * TCH], ps, ACT.Copy)

            def emit_wprep(hb):
                emit_wprep_dma(hb)
                for piece in range(6):
                    emit_wprep_piece(hb, piece)

            hs_tiles = [[None] * N_TC for _ in range(HB)]
            prev_hs_map = {}

            def emit_compute(hb, tcs=None, splits=1):
                wt = wt_tiles[hb]
                prev_hs, prev_end = prev_hs_map.get(hb, (None, TCH))
                W = TCH // splits
                for tc_i in (range(N_TC) if tcs is None else tcs):
                    pre = []
                    for g in range(3):
                        psg = mm_ps.tile([P, TCH], fp32,
                                         name=f"ps{hb}_{tc_i}_{g}", tag="mm")
                        for kd in range(KD):
                            nc.tensor.matmul(
                                psg,
                                wt[g][:, kd * P:(kd + 1) * P],
                                xt[kd][tc_i],
                                start=(kd == 0), stop=(kd == KD - 1))
                        pre.append(psg)

                    sf = gates.tile([P, TCH], fp32, name=f"sf{hb}_{tc_i}",
                                    tag="sf")
                    si = gates.tile([P, TCH], fp32, name=f"si{hb}_{tc_i}",
                                    tag="si")
                    sh = gates.tile([P, TCH], fp32, name=f"sh{hb}_{tc_i}",
                                    tag="sh")
                    gg = gates.tile([P, TCH], fp32, name=f"gg{hb}_{tc_i}",
                                    tag="gg")
                    den = gates.tile([P, TCH], fp32, name=f"den{hb}_{tc_i}",
                                     tag="den")
                    rec = gates.tile([P, TCH], fp32, name=f"rec{hb}_{tc_i}",
                                     tag="rec")
                    fg = gates.tile([P, TCH], fp32, name=f"fg{hb}_{tc_i}",
                                    tag="fg")
                    nv = gates.tile([P, TCH], fp32, name=f"nv{hb}_{tc_i}",
                                    tag="nv")
                    hs = hs_pool.tile([P, TCH], fp32, name=f"hs{hb}_{tc_i}",
                                      tag="hs")
                    for s in range(splits):
                        sl = slice(s * W, (s + 1) * W)
                        nc.scalar.activation(sf[:, sl], pre[0][:, sl],
                                             ACT.Sigmoid,
                                             bias=bf_t[:, hb:hb + 1])
                        nc.scalar.activation(si[:, sl], pre[1][:, sl],
                                             ACT.Sigmoid,
                                             bias=bi_t[:, hb:hb + 1])
                        nc.scalar.activation(sh[:, sl], pre[2][:, sl],
                                             ACT.Sigmoid,
                                             bias=bh_t[:, hb:hb + 1])
                        # g = max(Hp + bh + 0.5, sigmoid(Hp + bh))
                        nc.vector.scalar_tensor_tensor(
                            gg[:, sl], pre[2][:, sl], bhp5[:, hb:hb + 1],
                            sh[:, sl], op0=ALU.add, op1=ALU.max)
                        nc.vector.tensor_add(den[:, sl], sf[:, sl],
                                             si[:, sl])
                        nc.vector.reciprocal_approx_fast(rec[:, sl],
                                                         den[:, sl])
                        nc.vector.tensor_mul(fg[:, sl], sf[:, sl],
                                             rec[:, sl])
                        # nv = (f-1)*g (scan's op1=subtract adds (1-f)*g)
                        nc.vector.scalar_tensor_tensor(
                            nv[:, sl], fg[:, sl], 1.0, gg[:, sl],
                            op0=ALU.subtract, op1=ALU.mult)
                        init = (g0[:, hb:hb + 1]
                                if tc_i == 0 and s == 0
                                else prev_hs[:, prev_end - 1:prev_end])
                        nc.vector.tensor_tensor_scan(hs[:, sl], fg[:, sl],
                                                     nv[:, sl], init,
                                                     op0=ALU.mult,
                                                     op1=ALU.subtract)
                        prev_hs, prev_end = hs, sl.stop
                    hs_tiles[hb][tc_i] = hs
                prev_hs_map[hb] = (prev_hs, prev_end)

            def emit_out(hb, tcs=None):
                # DVE 32x32 block transpose + block-permuting DMA AP: the
                # in-block transpose happens on DVE, the block-position swap
                # happens in the DMA's access pattern (32-float runs).
                for tc_i in (range(N_TC) if tcs is None else tcs):
                    t0 = tc_i * TCH
                    hs = hs_tiles[hb][tc_i]
                    hst = ost_pool.tile([P, TCH], fp32,
                                        name=f"hst{hb}_{tc_i}", tag="ost")
                    nc.vector.transpose(hst, hs)
                    y_ap = y[1 + t0:1 + t0 + TCH, hb * P:(hb + 1) * P]
                    for A in range(4):
                        nc.sync.dma_start(
                            out=y_ap[:, A * 32:(A + 1) * 32].rearrange(
                                "(Bq u) v -> u Bq v", u=32),
                            in_=hst[A * 32:(A + 1) * 32, :].rearrange(
                                "u (Bq v) -> u Bq v", v=32))

            # ---- emission schedule (software-pipelined for PE density) ----
            # W-prep is emitted mid-way through the previous h-block so its
            # ACT copies drain before the next block's matmuls need them;
            # output transposes are delayed one h-block so their scan inputs
            # are long since ready when PE reaches them.
            emit_wprep(0)
            for tc_i in range(N_TC):
                emit_xT(tc_i)
                emit_compute(0, tcs=[tc_i])
                if tc_i == 0:
                    emit_wprep(1)
            for hb in range(1, HB):
                emit_compute(hb, tcs=[0])
                if hb + 1 < HB:
                    emit_wprep(hb + 1)
                if hb < HB - 1:
                    emit_compute(hb, tcs=[1, 2, 3])
                    emit_out(hb - 1)
                else:
                    emit_compute(hb, tcs=[1], splits=2)
                    emit_out(hb - 1, tcs=[0, 1])
                    emit_compute(hb, tcs=[2], splits=2)
                    emit_out(hb - 1, tcs=[2, 3])
                    emit_compute(hb, tcs=[3], splits=2)
                    emit_out(hb, tcs=[0, 1])
            emit_out(HB - 1, tcs=[2, 3])

    nc.compile()
    return nc


def _get_nc():
    if "nc" not in _cache:
        _cache["nc"] = _build_nc()
    return _cache["nc"]


def _run(inputs, trace=False, **kw):
    from concourse.bass_utils import run_bass_kernel_spmd

    nc = _get_nc()
    x = np.ascontiguousarray(inputs["x"], dtype=np.float32)
    h_0 = np.ascontiguousarray(inputs["h_0"], dtype=np.float32)
    shared = {
        "Wf": np.ascontiguousarray(inputs["Wf"], dtype=np.float32),
        "Wi": np.ascontiguousarray(inputs["Wi"], dtype=np.float32),
        "Wh": np.ascontiguousarray(inputs["Wh"], dtype=np.float32),
        "bf": np.ascontiguousarray(inputs["bf"], dtype=np.float32),
        "bi": np.ascontiguousarray(inputs["bi"], dtype=np.float32),
        "bh": np.ascontiguousarray(inputs["bh"], dtype=np.float32),
    }
    in_maps = []
    for b in range(B):
        m = {"x": x[b], "h0": h_0[b], **shared}
        in_maps.append(m)
    res = run_bass_kernel_spmd(nc, in_maps, list(range(N_CORES)),
                               trace=trace, **kw)
    out = np.stack([res.results[b]["y"] for b in range(B)], axis=0)
    return out, res


def kernel(**inputs) -> np.ndarray:
    out, _ = _run(inputs, trace=False)
    return out
